# revision 91
# baseline (speedup 1.0000x reference)
"""Trainium2 Bass kernel for causal multi-head attention (dense transformer block).

Problem: x[2,2048,1024] -> qkv proj -> 16-head causal attention (scale 1/sqrt(1024))
         -> out proj.  8 NeuronCores.

Sharding: core c handles batch b=c//4 and head-group r=c%4 (heads 4r..4r+3).

Structure (v2 — restructured for the cost-model roofline):
  - Q/K projection runs fp8-e4m3 DoubleRow (0.5 cyc/col): the host passes x
    and the q/k weight slices pre-packed in the fp8 DoubleRow layout
    (contraction chunk of 128 dm = 64 partitions x 2 planes), with the weights
    scaled by 32 so their magnitudes sit in fp8 normal range; the exp scale
    absorbs the 32*32 factor (2^-15).  V projection stays bf16.
  - Scores S^T[k,q] = K^T-chunk x Q^T fp8-DR as before; exps are batched in
    PAIRS of k-tiles: both scores land in one [128,2,512] PSUM group tile and
    a single activation exps them (halves the ~185ns/instruction ACT
    overhead; ACT is the bottleneck engine).
  - attnV is flipped: the P chunk [128k,128q] is the (free) stationary
    operand and [V|1] (65 cols) is the moving one, so each matmul costs 65
    cycles instead of up-to-512.  O accumulates q-major in PSUM [128q,4*65];
    column 64 of each chunk is the softmax denominator, which is now a
    per-partition scalar: normalize = reciprocal[128,2] + tensor_scalar_mul,
    then a [128,128] PE transpose per head-PAIR chunk writes OTs
    feature-major (both heads of the pair at once).
  - mask/identity constants come from the host (DMA) instead of gpsimd.
  - AllGather (bf16, groups of 4 cores sharing a batch) per 512-token pass;
    final pass gathers its two feature halves separately, with the first
    half's out-proj contributions accumulated into held PSUM banks as PE
    filler and keep-warm matmuls covering the last gather's p-state ramp.
  - qkv projection for block b+1 and deferred out-projections are emitted in
    small units between attention jobs to keep the PE fed under the exp wall.

kernel(**inputs) takes the FULL fp32 inputs and returns the FULL output.
"""

import sys

sys.path.insert(0, "/opt/trn_rl_repo")

import ml_dtypes
import numpy as np

import concourse.bass as bass
import concourse.bacc as bacc
import concourse.mybir as mybir
import concourse.tile as tile
from concourse.bass import ds, ts
from concourse.bass_utils import run_bass_kernel_spmd

F32 = mybir.dt.float32
BF16 = mybir.dt.bfloat16
FP8 = mybir.dt.float8e4

# ---------------------------------------------------------------- dims
BS, L, DM, H = 2, 2048, 1024, 16
HD = 64                      # head dim
NCORES = 8
GRP = 4                      # cores per batch group (head-parallel)
HLOC = H // GRP              # heads per core = 4
FLOC = HLOC * HD             # local features = 256
REPLICA_GROUPS = [[0, 1, 2, 3], [4, 5, 6, 7]]
WSCALE = 32.0                # host-side q/k weight scale (fp8 range)
EXP_SCALE = 1.0 / (WSCALE * WSCALE * float(np.sqrt(DM)))  # 2^-15


class Cfg:
    """Geometry (parametrized so a small config can be tested quickly)."""

    def __init__(self, L=L, DM=DM, hloc=HLOC, hd=HD, npass=4):
        self.L, self.DM, self.HLOC, self.HD, self.NPASS = L, DM, hloc, hd, npass
        self.FLOC = hloc * hd
        self.NT = L // 128           # 128-token tiles
        self.NB = L // 512           # 512-token blocks
        self.NDM = DM // 128         # dmodel chunks
        self.NDM2 = self.NDM // 2    # fp8 DR chunk slots
        self.PW = L // npass         # pass width (q columns per pass)
        self.NFT = self.FLOC // 128  # feature tiles for OTs (2)
        assert self.PW == 512 and self.FLOC % 128 == 0


def build_body(nc, cfg, x, x8, w8, wv, bq, bk, bv, wo, bo, maskm, identm, out,
               groups):
    """Emit the per-core program (Tile framework) for one iteration.

    `x` is the PRE-TRANSPOSED input [DM, L] fp32 (dm-major); `x8`/`w8` are the
    host-packed fp8 DoubleRow tensors [64, NDM, 2, L] / [64, NDM, 2, 2*FLOC].
    """
    NT, NB, NDM, NDM2, PW, NFT = cfg.NT, cfg.NB, cfg.NDM, cfg.NDM2, cfg.PW, cfg.NFT
    HLOCc, HDc, FLOCc = cfg.HLOC, cfg.HD, cfg.FLOC
    Lc, DMc = cfg.L, cfg.DM
    NPASS = cfg.NPASS
    tc = nc.tc

    with tc.tile_pool(name="const", bufs=1) as constp, \
         tc.tile_pool(name="persist", bufs=1) as pp:
        # ---------------- persistent SBUF tensors
        xT = pp.tile([128, NDM, Lc], BF16)                 # x^T bf16 (for V)
        # fp8 DoubleRow operands live on partitions 0..64 only: every DR
        # matmul runs at tile_position (0,0) — mixing row offsets between
        # 64-partition DR matmuls crashes the hardware.
        x8T = pp.tile([64, NDM, 2, Lc], FP8)               # x^T fp8 DR packed
        w8b = pp.tile([64, NDM, 2, 2 * FLOCc], FP8)        # [wq|wk] fp8 DR
        wvb = pp.tile([128, NDM, FLOCc], BF16)
        wob = pp.tile([128, NDM, FLOCc], BF16)
        # Q^T/K^T in fp8-e4m3 DoubleRow layout: partition block [32h,32h+32)
        # = head h; plane 0 = head dims 0..31, plane 1 = dims 32..63 (weight
        # columns are host-permuted to match).  Values are 32x-scaled.
        QT = pp.tile([128, 2, Lc], FP8)
        KT = pp.tile([128, 2, Lc], FP8)
        Vb = pp.tile([128, NT, HLOCc * (HDc + 1)], BF16)   # [V | ones] per token tile
        OTs = pp.tile([128, NFT, Lc], BF16)                # attention out^T (feature-major)

        # ---------------- PSUM pools for the whole kernel (8 banks):
        # stile [128,2,512]x2 = 4, po [128,260]x1 = 1, pt [128,2,128]bf16 x1
        # = 1, work [128,512]x2 = 2.  Opened before staging so no
        # pool-boundary barrier lands on the critical path.
        psum_cm = tc.tile_pool(name="psumS", bufs=2, space="PSUM")
        psum = psum_cm.__enter__()
        psum_po_cm = tc.tile_pool(name="psumPO", bufs=1, space="PSUM")
        psum_po = psum_po_cm.__enter__()
        psum_pt_cm = tc.tile_pool(name="psumPT", bufs=1, space="PSUM")
        psum_pt = psum_pt_cm.__enter__()
        psum2_cm = tc.tile_pool(name="psumW", bufs=2, space="PSUM")
        psum2 = psum2_cm.__enter__()

        # PE warmup: junk matmuls at the head so the p-state ramp happens on
        # dead time (the DMA-bound front), not on the first real matmuls.
        # Sized to end about when the first qk inputs land: an idle PE edge
        # resets the p-state and everything queued behind it gets costed slow.
        NWARM = 12
        wsrc_t = pp.tile([128, 512], BF16, name="wsrc_t")
        nc.vector.memset(wsrc_t, 0.25)
        wps = psum2.tile([128, 512], F32, tag="work", name="wps")
        for r in range(NWARM):
            nc.tensor.matmul(wps, wsrc_t[:, 0:128], wsrc_t,
                             start=(r == 0), stop=(r == NWARM - 1))
        wout_t = pp.tile([128, 512], F32, name="wout_t")
        nc.vector.tensor_copy(wout_t, wps)
        # pull the exp table load off the critical path
        nc.scalar.activation(wout_t[:, 0:2], wps[:, 0:2],
                             mybir.ActivationFunctionType.Exp, scale=1.0)



        # ---------------- constants
        maskM = constp.tile([128, 128], BF16)   # strict-upper -1e9 (k-row, q-col)
        ident = constp.tile([128, 128], BF16)   # identity
        ones_r = constp.tile([1, 128], BF16)
        negM = constp.tile([128, 128], BF16)    # all -1e9 (gap strip mask)
        bq_f = constp.tile([128, NFT], F32)
        bk_f = constp.tile([128, NFT], F32)
        bvb = constp.tile([1, FLOCc], BF16)
        bob = constp.tile([1, FLOCc], BF16)

        # ---------------- staging
        # HWDGE (sync) carries everything that needs no cast: fp8 x/w, masks,
        # biases.  The serial Pool SWDGE queue only does the f32->bf16 cast
        # loads (xT, wv, wo).  All transfers contend on the single DMA_ENGINES
        # device, so order = priority: tiny consts, then the q/k fp8 path
        # (first scores), then the V path (first attnV), then later blocks.
        # q/k fp8 path first (gates the first scores), then masks (first
        # scores are diagonal) and the q/k biases (their tensor_scalar_adds
        # gate the work-pool rotation).  Each sync dma pays ~625ns serial
        # HWDGE overhead and transfers contend FIFO on DMA_ENGINES, so order
        # is priority; a tiny first Pool dma (bv_st) delays the Pool queue's
        # first big transfer past x8-b0's claim.
        nc.sync.dma_start(w8b[:, :, :, 0:FLOCc], w8[:, :, :, 0:FLOCc])
        nc.sync.dma_start(x8T[:, :, :, ts(0, 512)], x8[:, :, :, ts(0, 512)])
        nc.sync.dma_start(w8b[:, :, :, FLOCc : 2 * FLOCc],
                          w8[:, :, :, FLOCc : 2 * FLOCc])
        nc.sync.dma_start(maskM, maskm)
        nc.sync.dma_start(ident, identm)
        nc.sync.dma_start(bq_f, bq.rearrange("(f p) -> p f", p=128))
        nc.sync.dma_start(bk_f, bk.rearrange("(f p) -> p f", p=128))
        bv_st = constp.tile([1, 2 * FLOCc], F32, name="bv_st")
        nc.gpsimd.dma_start(bv_st[:, 0:FLOCc], bv.rearrange("(a b) -> a b", a=1))
        nc.gpsimd.dma_start(bv_st[:, FLOCc : 2 * FLOCc], bo.rearrange("(a b) -> a b", a=1))
        nc.vector.memset(ones_r, 1.0)
        nc.vector.memset(negM, -1e9)
        nc.vector.tensor_copy(bvb, bv_st[:, 0:FLOCc])
        nc.vector.tensor_copy(bob, bv_st[:, FLOCc : 2 * FLOCc])
        nc.vector.memset(
            Vb.rearrange("p t (h u) -> p t h u", u=HDc + 1)[:, :, :, HDc : HDc + 1], 1.0
        )

        xTv = x.rearrange("(c p) t -> p c t", p=128)
        nc.gpsimd.dma_start(wvb, wv.rearrange("(c p) f -> p c f", p=128))
        nc.gpsimd.dma_start(xT[:, :, ts(0, 512)], xTv[:, :, ts(0, 512)])
        for b in range(1, NB):
            nc.sync.dma_start(x8T[:, :, :, ts(b, 512)], x8[:, :, :, ts(b, 512)])
            nc.gpsimd.dma_start(xT[:, :, ts(b, 512)], xTv[:, :, ts(b, 512)])
        nc.gpsimd.dma_start(wob, wo.rearrange("(c p) f -> p c f", p=128))

        # ---------------- qkv projection units for one 512-token block.
        # q/k run fp8 DoubleRow: stationary w8 slice [64,2,128], moving x8
        # slice [64,2,512] -> psum [128,512] at 0.5 cyc/col.  PSUM->SBUF
        # copies ride the DVE (tensor_scalar_add applies the bias and casts
        # to the fp8 QT/KT layout).
        def qk_units(tb, pl, woff, dest, bias):
            st = {}
            def u_mm(n):
                def f():
                    if n == 0:
                        st["t"] = psum2.tile([128, 512], F32, tag="work", name="qs")
                    for c in range(2 * n, 2 * n + 2):
                        nc.tensor.matmul(
                            st["t"],
                            w8b[:, c, :, woff + pl * 128 : woff + pl * 128 + 128],
                            x8T[:, c, :, ts(tb, 512)],
                            start=(c == 0), stop=(c == NDM - 1),
                            perf_mode=mybir.MatmulPerfMode.DoubleRow,
                            tile_position=(0, 0),
                        )
                    if n == NDM2 - 1:
                        nc.vector.tensor_scalar_add(dest[:, pl, ts(tb, 512)],
                                                    st["t"], bias[:, pl : pl + 1])
                return f
            return [u_mm(n) for n in range(NDM2)]

        def v_units(tt):
            st = {}
            def u_mm(cg):
                def f():
                    if cg == 0:
                        st["pf"] = psum2.tile([128, 512], F32, tag="work",
                                              name="psv_full")
                    psv = st["pf"][:, 0:FLOCc]
                    for c in range(2 * cg, 2 * cg + 2):
                        nc.tensor.matmul(
                            psv, xT[:, c, ts(tt, 128)], wvb[:, c, :],
                            start=(c == 0), stop=False,
                        )
                    if cg == NDM // 2 - 1:
                        nc.tensor.matmul(psv, ones_r, bvb, start=False, stop=True)
                        nc.vector.tensor_copy(
                            Vb[:, tt, :].rearrange("p (h u) -> p h u", u=HDc + 1)[:, :, 0:HDc],
                            psv.rearrange("p (h d) -> p h d", d=HDc),
                        )
                return f
            return [u_mm(cg) for cg in range(NDM // 2)]

        def qkv_units(tb):
            units = []
            for pl in range(2):
                units.extend(qk_units(tb, pl, 0, QT, bq_f))
                units.extend(qk_units(tb, pl, FLOCc, KT, bk_f))
            for tt in range(tb * 4, tb * 4 + 4):
                units.extend(v_units(tt))
            return units

        def emit_qkv_qk(tb):
            for pl in range(2):
                for f in qk_units(tb, pl, 0, QT, bq_f):
                    f()
                for f in qk_units(tb, pl, FLOCc, KT, bk_f):
                    f()

        # ---------------- attention + allgather + out projection
        with tc.tile_pool(name="pbuf", bufs=4) as pbp, \
             tc.tile_pool(name="onrm", bufs=2) as onp, \
             tc.tile_pool(name="rcp", bufs=2) as rcp, \
             tc.tile_pool(name="of", bufs=3) as ofp, \
             tc.tile_pool(name="osb", bufs=3) as osbp, \
             tc.tile_pool(name="dram", bufs=2, space="DRAM") as dramp:

            # ---- allgather + out-projection machinery (as in v1)
            def emit_ag(p):
                """AllGather of pass p's OTs block + gathered-feature loads.
                Everything rides the DMA queues, so it cannot park the
                in-order PE queue."""
                q0 = p * PW
                ag_in = dramp.tile([NFT * 128, 512], BF16, tag="agin", name="ag_in")
                ag_out = dramp.tile([GRP * NFT * 128, 512], BF16, tag="agout", name="ag_out")
                for t in range(NFT):
                    nc.sync.dma_start(ag_in[ts(t, 128), :], OTs[:, t, ds(q0, 512)])
                nc.gpsimd.collective_compute(
                    "AllGather",
                    mybir.AluOpType.bypass,
                    ins=[ag_in.opt()],
                    outs=[ag_out.opt()],
                    replica_groups=groups,
                )
                OF = ofp.tile([128, NDM, 512], BF16, tag="of", name="OF")
                agv = ag_out.rearrange("(c p) q -> p c q", p=128)
                for c2 in range(NDM // 2):
                    nc.sync.dma_start(OF[:, 2 * c2 : 2 * c2 + 2, :],
                                      agv[:, 2 * c2 : 2 * c2 + 2, :])
                return OF

            def outproj_units(p, OF):
                q0 = p * PW
                st = {}
                outv = out[ds(q0, 512), :].rearrange("(t p) f -> p t f", p=128)
                def u_ttl(ttl):
                    def f():
                        if ttl == 0:
                            st["osb"] = osbp.tile([128, 4, FLOCc], F32,
                                                  tag="osb", name="osb")
                        pout_full = psum2.tile([128, 512], F32, tag="work",
                                               name="pout_full")
                        pout = pout_full[:, 0:FLOCc]
                        for c in range(NDM):
                            nc.tensor.matmul(
                                pout, OF[:, c, ts(ttl, 128)], wob[:, c, :],
                                start=(c == 0), stop=False,
                            )
                        nc.tensor.matmul(pout, ones_r, bob, start=False, stop=True)
                        nc.vector.tensor_copy(st["osb"][:, ttl, :], pout)
                        if ttl == 1:
                            nc.sync.dma_start(outv[:, 0:2, :], st["osb"][:, 0:2, :])
                        elif ttl == 3:
                            nc.sync.dma_start(outv[:, 2:4, :], st["osb"][:, 2:4, :])
                    return f
                return [u_ttl(t) for t in range(4)]

            # ---- half-feature AllGather for the FINAL pass.  Gathered chunk
            # r of half ft maps to wob chunk 2r+ft.
            def emit_ag_half(p, fth, ag_in_pre=None):
                q0 = p * PW
                if ag_in_pre is None:
                    ag_in_h = dramp.tile([128, 512], BF16, tag="aginh", name="ag_in_h")
                    nc.sync.dma_start(ag_in_h, OTs[:, fth, ds(q0, 512)])
                else:
                    ag_in_h = ag_in_pre
                ag_out_h = dramp.tile([GRP * 128, 512], BF16, tag="agouth", name="ag_out_h")
                nc.gpsimd.collective_compute(
                    "AllGather",
                    mybir.AluOpType.bypass,
                    ins=[ag_in_h.opt()],
                    outs=[ag_out_h.opt()],
                    replica_groups=groups,
                )
                OFh = ofp.tile([128, GRP, 512], BF16, tag="ofh", name="OFh")
                agvh = ag_out_h.rearrange("(c p) q -> p c q", p=128)
                for c2 in range(GRP // 2):
                    nc.sync.dma_start(OFh[:, 2 * c2 : 2 * c2 + 2, :],
                                      agvh[:, 2 * c2 : 2 * c2 + 2, :])
                return OFh

            # ---------------- the attention job stream
            # Per pass p: 4 heads x (2p+2) k-tile PAIRS.  Per pair: both
            # scores into one stile group, one batched exp, chunked attnV
            # (P stationary, V moving).  attnV runs ONE PAIR BEHIND its exp
            # so the in-order PE queue never parks on the activation engine
            # (a PE idle edge resets the p-state and doubles the cost of the
            # burst queued behind it).  Normalize per (head, chunk-pair)
            # right after the chunk's accumulation stops.
            emit_qkv_qk(0)
            OF_all = {}
            state = {"onorm": None, "po": None, "pt": None}

            def emit_scores_pair(h, i0, qs0):
                Sg = psum.tile([128, 2, 512], F32, tag="stile", name="Sg")
                al0 = max(0, 128 * i0 - qs0)
                al1 = max(0, 128 * (i0 + 1) - qs0)
                if al1 > al0:
                    # the pair-exp covers [al0:512) of BOTH halves; zero the
                    # never-matmul-written gap of half 1 (the exp'd values
                    # are not consumed, but the bytes must be initialized)
                    nc.vector.memset(Sg[:, 1, ds(al0, al1 - al0)], 0.0)
                for j in range(2):
                    i = i0 + j
                    al = max(0, 128 * i - qs0)
                    is_diag = 128 * i >= qs0
                    nc.tensor.matmul(
                        Sg[:, j, ds(al, 512 - al)],
                        KT[32 * h : 32 * h + 32, :, ts(i, 128)],
                        QT[32 * h : 32 * h + 32, :, ds(qs0 + al, 512 - al)],
                        start=True, stop=not is_diag,
                        skip_group_check=True,
                        perf_mode=mybir.MatmulPerfMode.DoubleRow,
                        tile_position=(32 * h, 0),
                    )
                    if is_diag:
                        nc.tensor.matmul(
                            Sg[:, j, ds(al, 128)], maskM, ident,
                            start=False, stop=True,
                            skip_group_check=True,
                        )
                return Sg

            def emit_exp(Sg, i0, qs0):
                al0 = max(0, 128 * i0 - qs0)
                P2 = pbp.tile([128, 2, 512], BF16, tag="ptile", name="P2")
                nc.scalar.activation(
                    P2[:, :, ds(al0, 512 - al0)],
                    Sg[:, :, ds(al0, 512 - al0)],
                    mybir.ActivationFunctionType.Exp,
                    scale=float(EXP_SCALE),
                )
                return P2

            def emit_attnv(P2, h, i0, p):
                if i0 == 0:
                    # fresh accumulator for this head (cols 260:512 are the
                    # junk-ballast area, never read)
                    state["po"] = psum_po.tile([128, 512], F32, tag="po",
                                               name="po")
                    if h % 2 == 0:
                        state["pt"] = psum_pt.tile([128, 2, 128], BF16,
                                                   tag="pt", name="pt")
                po = state["po"]
                for j in range(2):
                    i = i0 + j
                    for qc in range(4):
                        if i > 4 * p + qc:
                            continue
                        # start only on the bank's FIRST matmul: start=True
                        # marks the whole 2KB zero-region pending-zero, so a
                        # second start would wipe other chunks' partials
                        nc.tensor.matmul(
                            po[:, ds(65 * qc, 65)],
                            P2[:, j, ts(qc, 128)],
                            Vb[:, i, ds((HDc + 1) * h, HDc + 1)],
                            start=(i == 0 and qc == 0), stop=(i == 4 * p + qc),
                            skip_group_check=True,
                        )

            def emit_norm(h, p, m):
                """Normalize + transpose chunks qc=2m,2m+1 of head h, pass p
                (their attnV accumulation just stopped)."""
                po = state["po"]
                hf, hp = h // 2, h % 2
                qs0 = p * PW
                if hp == 0 and m == 0:
                    state["onorm"] = onp.tile([128, 4, 128], BF16,
                                              tag="onorm", name="onorm")
                onorm = state["onorm"]
                pov = po[:, 0 : 4 * (HDc + 1)].rearrange("p (q c) -> p q c",
                                                         c=HDc + 1)
                last_h = (p == NPASS - 1 and h == HLOCc - 1)
                if not last_h and m == 0:
                    # defer: m=1's flush normalizes all 4 chunks with ONE
                    # reciprocal, halving the DVE hops (the last head stays
                    # per-m so the tail chain isn't lengthened)
                    return
                if last_h:
                    qcs = range(2 * m, 2 * m + 2)
                    roff = 2 * m
                    rec = rcp.tile([128, 2], F32, tag="rec", name="rec")
                    nc.vector.reciprocal(rec, pov[:, 2 * m : 2 * m + 2, HDc])
                else:
                    qcs = range(4)
                    roff = 0
                    rec = rcp.tile([128, 4], F32, tag="rec", name="rec4",
                                   bufs=2)
                    nc.vector.reciprocal(rec, pov[:, :, HDc])
                for qc in qcs:
                    nc.vector.tensor_scalar_mul(
                        onorm[:, qc, ds(64 * hp, 64)],
                        po[:, ds(65 * qc, HDc)],
                        rec[:, qc - roff : qc - roff + 1],
                    )
                if hp == 1:
                    # merged path: transpose qc pairs, then ONE [128,2,128]
                    # copy per pair (pt slot order == OTs column order); the
                    # last head streams each half to the ag buffer with ONE
                    # [128,256] store
                    pt = state["pt"]
                    for half in ([m] if last_h else range(2)):
                        for qc in (2 * half, 2 * half + 1):
                            nc.tensor.transpose(
                                pt[:, qc % 2, :], onorm[:, qc, :], ident)
                        nc.vector.tensor_copy(
                            OTs[:, hf, ds(qs0 + 256 * half, 256)],
                            pt,
                        )
                        if last_h:
                            nc.sync.dma_start(
                                ag1_piece["t"][:, ds(256 * half, 256)],
                                OTs[:, hf, ds(qs0 + 256 * half, 256)],
                            )

            # deferred out-proj unit placement: op_sched[p] = list of
            # (pass_to_project, OF-getter) unit lists woven into pass p.
            op_units_all = {}

            for p in range(NPASS):
                qs0 = p * PW
                npairs = 2 * p + 2
                jobs = [(h, 2 * jj) for h in range(HLOCc) for jj in range(npairs)]

                # filler units for this pass, balanced so per-pass PE work
                # stays under the pass's exp-wall: back-loaded weave carries
                # qk for block p+1 (needed at pass p+1 start) plus deferred
                # out-projections (passes 0,1 -> pass 2; pass 2 -> pass 3);
                # v-units for block p ride THIS pass's early jobs (their
                # tiles are consumed mid-pass).
                units = []
                if p < NPASS - 1:
                    for pl in range(2):
                        units.extend(qk_units(p + 1, pl, 0, QT, bq_f))
                        units.extend(qk_units(p + 1, pl, FLOCc, KT, bk_f))
                if p == 2:
                    units.extend(outproj_units(0, OF_all[0]))
                    units.extend(outproj_units(1, OF_all[1]))
                if p == 3:
                    units.extend(outproj_units(2, OF_all[2]))
                nu = len(units)

                # v-units for block p woven into this pass's early jobs
                # (tile 4p+tt is first consumed at job 2p+tt//2+1)
                v_at = {}
                if p == 0:
                    v_at = {0: v_units(0) + v_units(1),
                            1: v_units(2) + v_units(3)}
                else:
                    vunits = []
                    for tt in range(4 * p, 4 * p + 4):
                        vunits.extend(v_units(tt))
                    nv = len(vunits)
                    vspan = 2 * p + 2
                    for jx in range(vspan):
                        v_at[jx] = vunits[jx * nv // vspan:
                                          (jx + 1) * nv // vspan]

                # final pass schedule markers
                sp_half = 2 * npairs   # first job of h=2: ft0 gather point
                OFA = None
                ofa_tiles = None
                # OFA partial out-proj (one ttl per job) rides h3's stream,
                # after the ft0 gather has certainly landed.  The weave window
                # ends before it so work-pool rotation can't alias the held
                # pout banks.
                ofa_at = {}
                if p == NPASS - 1:
                    ofa_at = {3 * npairs + 1 + t: t for t in range(4)}
                    whi = 3 * npairs + 1
                    # the woven out-proj for pass 2 reads OF data whose
                    # gather rides early in this pass: start the weave late
                    k0 = 10
                else:
                    whi = len(jobs)
                    k0 = len(jobs) // 5

                # junk-ballast matmuls per job: fill the exp-wait slack in
                # ACT-bound passes so the PE p-state never resets (writes the
                # never-read cols 260:512 of the current po bank).
                njunk = (0, 0, 1, 3)[p]

                def emit_junk(n):
                    po = state["po"]
                    if po is None:
                        return
                    # start=False: a start here would mark the whole po bank
                    # pending-zero and wipe the live attnV partials
                    for _ in range(n):
                        nc.tensor.matmul(po[:, 260:512], wsrc_t[:, 0:128],
                                         wsrc_t[:, 0:252], start=False,
                                         stop=False, skip_group_check=True)

                # pending attnV queue: attnV for pair j is emitted at job j+1.
                # Norms lag one more job so their PE transposes (gated on the
                # DVE normalize chain) can't park the next head's scores.
                Pq = []
                Nq = []
                ag1_piece = {}

                def flush_norm():
                    hp_, m_ = Nq.pop(0)
                    emit_norm(hp_, p, m_)

                def flush_attnv(norm_eager=False):
                    P2p, hp_, i0p = Pq.pop(0)
                    emit_attnv(P2p, hp_, i0p, p)
                    if i0p >= 4 * p:
                        Nq.append((hp_, (i0p - 4 * p) // 2))
                        if norm_eager:
                            while Nq:
                                flush_norm()

                if p == NPASS - 1:
                    ag1_piece["t"] = dramp.tile([128, 512], BF16, tag="aginh",
                                                name="ag_in_h1")

                Sq = [emit_scores_pair(*jobs[0], qs0)] if jobs else []
                for idx, (h, i0) in enumerate(jobs):
                    jj = i0 // 2
                    Sg = Sq.pop(0)
                    if idx + 1 < len(jobs):
                        Sq.append(emit_scores_pair(*jobs[idx + 1], qs0))
                    P2 = emit_exp(Sg, i0, qs0)
                    if idx in v_at:
                        for f in v_at[idx]:
                            f()
                    # weave filler units over jobs [k0, whi), back-loaded
                    if nu:
                        lo = min(max(0, idx - k0), whi - k0)
                        hi = min(max(0, idx + 1 - k0), whi - k0)
                        for u in range(lo * nu // (whi - k0),
                                       hi * nu // (whi - k0)):
                            units[u]()
                    # final pass: OFA partial accumulation as PE filler
                    if idx in ofa_at:
                        ttl = ofa_at[idx]
                        if ofa_tiles is None:
                            ofa_tiles = [
                                psum2.tile([128, 512], F32, tag="work",
                                           name=f"pout{half}")
                                for half in range(2)
                            ]
                        pout = ofa_tiles[ttl // 2][:, (ttl % 2) * FLOCc:
                                                   (ttl % 2) * FLOCc + FLOCc]
                        for r in range(GRP):
                            # one start per held bank (ttl%2==1 shares the
                            # ttl%2==0 bank; its bytes are pending from that
                            # bank's start)
                            nc.tensor.matmul(
                                pout, OFA[:, r, ts(ttl, 128)],
                                wob[:, 2 * r, :],
                                start=(r == 0 and ttl % 2 == 0), stop=False,
                                skip_group_check=True,
                            )
                    if Nq:
                        flush_norm()
                    # ft0 half-gather only after h1's last norm has flushed
                    # (its transposes write the OTs block the gather reads)
                    if p == NPASS - 1 and idx == sp_half + 1:
                        OFA = emit_ag_half(p, 0)
                    emit_junk(njunk)
                    Pq.append((P2, h, i0))
                    # final head of the final pass flushes eagerly (no lag):
                    # its normalizes gate the last gather, so pulling them
                    # earlier shortens the kernel tail.
                    eager = (p == NPASS - 1 and h == HLOCc - 1)
                    while len(Pq) > (0 if eager else 1):
                        flush_attnv(norm_eager=eager)
                while Pq:
                    flush_attnv(norm_eager=True)
                while Nq:
                    flush_norm()

                if p < NPASS - 1:
                    OF_all[p] = emit_ag(p)
                else:
                    # ---- tail: gather second feature half, finish out-proj
                    OFB = emit_ag_half(p, 1, ag_in_pre=ag1_piece["t"])
                    # keep-warm junk (into a free stile bank) so the PE
                    # p-state survives the gather
                    wjunk = psum.tile([128, 2, 512], F32, tag="stile",
                                      name="wjunk")
                    for r in range(50):
                        nc.tensor.matmul(wjunk[:, 0, :], wsrc_t[:, 0:128],
                                         wsrc_t, start=(r == 0), stop=(r == 49),
                                         skip_group_check=True)
                    # finer-grained junk right before the OFB matmuls so the
                    # p-state stays warm up to the moment the gather lands
                    for r in range(10):
                        nc.tensor.matmul(wjunk[:, 1, 0:128], wsrc_t[:, 0:128],
                                         wsrc_t[:, 0:128], start=True,
                                         stop=True, skip_group_check=True)
                    outv = out[ds(qs0, 512), :].rearrange("(t p) f -> p t f", p=128)
                    osb = osbp.tile([128, 4, FLOCc], F32, tag="osb", name="osbF")
                    def poutat(ttl):
                        return ofa_tiles[ttl // 2][:, (ttl % 2) * FLOCc:
                                                   (ttl % 2) * FLOCc + FLOCc]
                    # r-outer: chunks 0,1 only need the first gathered load
                    for r in range(GRP):
                        for ttl in range(4):
                            nc.tensor.matmul(
                                poutat(ttl), OFB[:, r, ts(ttl, 128)],
                                wob[:, 2 * r + 1, :],
                                start=False, stop=False,
                                skip_group_check=True,
                            )
                    for ttl in range(4):
                        pout = poutat(ttl)
                        nc.tensor.matmul(pout, ones_r, bob, start=False,
                                         stop=True, skip_group_check=True)
                        nc.vector.tensor_copy(osb[:, ttl, :], pout)
                        if ttl == 1:
                            nc.sync.dma_start(outv[:, 0:2, :], osb[:, 0:2, :])
                        elif ttl == 3:
                            nc.sync.dma_start(outv[:, 2:4, :], osb[:, 2:4, :])

        psum2_cm.__exit__(None, None, None)
        psum_pt_cm.__exit__(None, None, None)
        psum_po_cm.__exit__(None, None, None)
        psum_cm.__exit__(None, None, None)


def make_program(cfg=None, groups=None, unroll=1):
    cfg = cfg or Cfg()
    groups = groups or REPLICA_GROUPS
    nc = bacc.Bacc("TRN2", target_bir_lowering=False, debug=False, num_devices=NCORES)
    x = nc.dram_tensor("x", [cfg.DM, cfg.L], F32, kind="ExternalInput").ap()
    x8 = nc.dram_tensor("x8", [64, cfg.NDM, 2, cfg.L], FP8, kind="ExternalInput").ap()
    w8 = nc.dram_tensor("w8", [64, cfg.NDM, 2, 2 * cfg.FLOC], FP8,
                        kind="ExternalInput").ap()
    wv = nc.dram_tensor("wv", [cfg.DM, cfg.FLOC], F32, kind="ExternalInput").ap()
    bq = nc.dram_tensor("bq", [cfg.FLOC], F32, kind="ExternalInput").ap()
    bk = nc.dram_tensor("bk", [cfg.FLOC], F32, kind="ExternalInput").ap()
    bv = nc.dram_tensor("bv", [cfg.FLOC], F32, kind="ExternalInput").ap()
    wo = nc.dram_tensor("wo", [cfg.DM, cfg.FLOC], F32, kind="ExternalInput").ap()
    bo = nc.dram_tensor("bo", [cfg.FLOC], F32, kind="ExternalInput").ap()
    maskm = nc.dram_tensor("maskm", [128, 128], BF16, kind="ExternalInput").ap()
    identm = nc.dram_tensor("identm", [128, 128], BF16, kind="ExternalInput").ap()
    out = nc.dram_tensor("out", [cfg.L, cfg.FLOC], F32, kind="ExternalOutput").ap()
    with tile.TileContext(nc) as tc:
        nc.tc = tc
        for _ in range(unroll):
            build_body(nc, cfg, x, x8, w8, wv, bq, bk, bv, wo, bo,
                       maskm, identm, out, groups)
    nc.compile()
    return nc


def _pack_dr(a):
    """[DM, N] -> fp8 DoubleRow host layout [64, NDM, 2, N].

    dm index d = 128*c + 64*pl + r64 maps to partition r64, chunk c,
    plane pl (all DR matmuls run on partitions 0..64 at tile_position 0)."""
    DMf, N = a.shape
    ndm = DMf // 128
    ar = a.reshape(ndm, 2, 64, N)              # [c, pl, r64, N]
    ar = np.transpose(ar, (2, 0, 1, 3))        # [r64, c, pl, N]
    return np.ascontiguousarray(
        ar.reshape(64, ndm, 2, N).astype(ml_dtypes.float8_e4m3))


def shard_inputs(x, w_qkv, b_qkv, w_out, b_out, cfg=None):
    """Full inputs -> list of 8 per-core input dicts."""
    cfg = cfg or Cfg()
    FL = cfg.FLOC
    DMF = cfg.DM
    in_maps = []
    xt = [np.ascontiguousarray(x[b].T) for b in range(BS)]
    x8t = [_pack_dr(xt[b]) for b in range(BS)]
    maskm = np.triu(np.full((128, 128), -1e9, np.float32), 1).astype(ml_dtypes.bfloat16)
    identm = np.eye(128, dtype=np.float32).astype(ml_dtypes.bfloat16)
    # permutation for the fp8 DoubleRow Q/K layout: new col j (j<128: plane 0)
    # = head (j//32), dim (j%32) + 32*plane
    perm = np.array([64 * ((j % 128) // 32) + 32 * (j // 128) + (j % 32)
                     for j in range(FL)])
    for c in range(NCORES):
        b, r = divmod(c, GRP)
        q0 = r * FL
        wq = WSCALE * w_qkv[:, q0 : q0 + FL][:, perm]
        wk = WSCALE * w_qkv[:, DMF + q0 : DMF + q0 + FL][:, perm]
        in_maps.append({
            "x": xt[b],
            "x8": x8t[b],
            "w8": _pack_dr(np.concatenate([wq, wk], axis=1)),
            "wv": np.ascontiguousarray(w_qkv[:, 2 * DMF + q0 : 2 * DMF + q0 + FL]),
            "bq": np.ascontiguousarray(WSCALE * b_qkv[q0 : q0 + FL][perm]),
            "bk": np.ascontiguousarray(WSCALE * b_qkv[DMF + q0 : DMF + q0 + FL][perm]),
            "bv": np.ascontiguousarray(b_qkv[2 * DMF + q0 : 2 * DMF + q0 + FL]),
            "wo": np.ascontiguousarray(w_out[:, q0 : q0 + FL]),
            "bo": np.ascontiguousarray(b_out[q0 : q0 + FL]),
            "maskm": maskm,
            "identm": identm,
        })
    return in_maps


def gather_output(results, cfg=None):
    cfg = cfg or Cfg()
    FL = cfg.FLOC
    out = np.empty((BS, cfg.L, cfg.DM), np.float32)
    for c in range(NCORES):
        b, r = divmod(c, GRP)
        out[b, :, r * FL : (r + 1) * FL] = results[c]["out"]
    return out


_PROGRAM = None


def _get_program():
    global _PROGRAM
    if _PROGRAM is None:
        _PROGRAM = make_program()
    return _PROGRAM


def kernel(x, w_qkv, b_qkv, w_out, b_out):
    x = np.asarray(x, np.float32)
    w_qkv = np.asarray(w_qkv, np.float32)
    b_qkv = np.asarray(b_qkv, np.float32)
    w_out = np.asarray(w_out, np.float32)
    b_out = np.asarray(b_out, np.float32)
    nc = _get_program()
    in_maps = shard_inputs(x, w_qkv, b_qkv, w_out, b_out)
    res = run_bass_kernel_spmd(nc, in_maps, list(range(NCORES)))
    return gather_output(res.results)
